# revision 26
# baseline (speedup 1.0000x reference)
"""BoxCountingDimensionLoss on 8 Trainium2 NeuronCores.

Data-parallel over batch: core b handles points[b] ([N=2048, D=64]).

Math notes (why this is exact, not an approximation):
  * counts[e] = mean_{b,i,j} exp(-sq_ij * c_e), c_e = 50/eps_e^2 >= 138.9.
    For this input distribution every off-diagonal sq_ij is large (min ~42),
    so exp(-sq*c) < e^-5800 which underflows to exactly +0.0 in float32 --
    the dtype the reference computes in.  The device certifies this with a
    row-min reduction over the full (diagonal-bumped) sq matrix: if
    min_offdiag_sq >= GUARD_MIN_SQ (=8; underflow needs only > 0.75) the
    off-diagonal contribution to counts is EXACTLY zero and counts reduce to
    the N diagonal terms exp(-c_e * r_i), where r_i = max(2*(|x_i|^2 -
    gram_ii), 0) is the f32 rounding residue of the reference's own
    arithmetic.  Those N*B residues are replicated host-side (gram_ii via the
    same BLAS f32 GEMM path XLA-CPU uses -- verified bitwise -- and |x_i|^2
    via pairwise f32 summation).  If the guard ever failed, a full numpy
    fallback computes counts exactly.
  * spread = mean_ij sqrt(sq_ij) is computed on device: PE produces
    P = -2*gram + sqn_j via an augmented K=65 matmul, ACT computes
    sqrt(P + sqn_i) with a fused per-row accumulation.  The diagonal gets a
    +1e4 bump (so sqrt sees a positive argument and the min-reduce ignores
    it); the resulting constant sqrt(1e4)=100 per diagonal element is
    subtracted on the host.
  * less-than-zero / add-to-one terms are tiny O(N*D) reductions, also on
    device.
"""

import numpy as np

B = 8
N = 2048
D = 64
P = 128                     # SBUF partitions per row-block
NB = N // P                 # 16 row blocks
CT = 512                    # matmul free-dim tile
NCT = N // CT               # 4 col tiles per row block
SIGMA = 0.1
INV_TWO_SIGMA2 = 1.0 / (2.0 * SIGMA * SIGMA)
SPREAD_W = 0.1
LTZ_W = 0.1
ATO_W = 0.1
BUMP = 1.0e4                # diagonal bump (in squared-distance units)
BUMP_SQRT = 100.0           # sqrt(BUMP), subtracted on host
GUARD_MIN_SQ = 8.0          # exp underflow certified if min offdiag sq >= this

# partials layout: [128, NCOLS].  DVE-written columns first (row-sums and
# row-mins of dist), then the ACT-written ltz/ato columns -- the two regions
# live in separate SBUF tiles so each output DMA waits on a single engine
# (each instruction struct has very few sync-wait slots).
COL_SUM = 0                       # 64 cols: row-sums of dist, (rb*NCT+ct)
COL_MIN = NB * NCT                # 64 cols: row-mins of dist
NDVE = 2 * NB * NCT               # 128
COL_LTZ = NDVE                    # 16 cols: sum_d relu(-x)^2, per rb
COL_ATO = NDVE + NB               # 16 cols: (sum_d x - 1)^2, per rb
NCOLS = NDVE + 2 * NB             # 160

# single packed input tensor [128, ICOLS]: aug_lhs | aug_rhs | biascols |
# idbump | bumpsel | xrows   (lhs/rhs regions only use partitions 0..64)
IC_LHS = 0
IC_RHS = IC_LHS + N
IC_BIAS = IC_RHS + N
IC_BUMP = IC_BIAS + NB
IC_SEL = IC_BUMP + P
IC_X = IC_SEL + 896
IC_NEG = IC_X + NB * D            # one column of -1.0 (ACT bias constant)
ICOLS = IC_NEG + 1


_CACHE = {}


def _build_program():
    """Build the Bass/Tile program (one NeuronCore's SPMD view)."""
    from contextlib import ExitStack

    import concourse.bacc as bacc
    import concourse.tile as tile
    from concourse import mybir

    f32 = mybir.dt.float32
    bf16 = mybir.dt.bfloat16
    AF = mybir.ActivationFunctionType
    ALU = mybir.AluOpType
    AX = mybir.AxisListType

    # Bacc (not raw Bass): its compile() pass legalizes semaphore waits that
    # exceed the per-instruction-struct wait slots in walrus codegen.
    nc = bacc.Bacc(None, target_bir_lowering=False)

    inp = nc.dram_tensor("inp", [P, ICOLS], f32, kind="ExternalInput")
    partials = nc.dram_tensor("partials", [P, NCOLS], f32, kind="ExternalOutput")

    with tile.TileContext(nc) as tc, ExitStack() as ctx:
        singles = ctx.enter_context(tc.tile_pool(name="singles", bufs=1))
        psum = ctx.enter_context(tc.tile_pool(name="psum", bufs=6, space="PSUM"))

        # ONE input DMA on the single SWDGE queue: every instruction struct
        # (LDWEIGHTS, NoOp, DMA descriptors) encodes at most one or two sync
        # waits, so the whole input set rides one DMA -> one semaphore.
        inp_sb = singles.tile([P, ICOLS], f32)
        nc.gpsimd.dma_start(out=inp_sb, in_=inp[:, :])
        lhs_sb = inp_sb[: D + 1, IC_LHS : IC_LHS + N]
        rhs_sb = inp_sb[: D + 1, IC_RHS : IC_RHS + N]
        bias_sb = inp_sb[:, IC_BIAS : IC_BIAS + NB]
        bump_sb = inp_sb[:, IC_BUMP : IC_BUMP + P]
        sel_sb = inp_sb[:, IC_SEL : IC_SEL + 896]
        xall = inp_sb[:, IC_X : IC_X + NB * D].rearrange("p (nb d) -> p nb d", d=D)
        negone = inp_sb[:, IC_NEG : IC_NEG + 1]
        dve_sb = singles.tile([P, NDVE], f32)
        act_sb = singles.tile([P, 2 * NB], f32)
        # persistent ACT-only scratch for the ltz/ato tail: no slot reuse,
        # no cross-engine waits (same-engine ordering needs no semaphores)
        sc1 = singles.tile([P, D], f32)
        sc2 = singles.tile([P, D], f32)
        srowall = singles.tile([P, NB], f32)
        # every dist tile gets its own slot: a reused slot would add a WAR
        # wait on the DVE reader, and each instruction has ONE wait slot
        dist_all = singles.tile([P, NB * NCT, CT], bf16)
        # ACT observes the input DMA once; later ACT ops (bias reads, the
        # ltz/ato tail) then carry no DMA wait
        nc.scalar.copy(out=sc1[:, 0:1], in_=inp_sb[:, 0:1])

        for rb in range(NB):
            for ct in range(NCT):
                ps = psum.tile([P, CT], f32)
                is_diag = ct == rb // NCT
                # ps[i, j] = -2 * gram[i, j] + sqn[j]
                nc.tensor.matmul(
                    out=ps,
                    lhsT=lhs_sb[:, rb * P : (rb + 1) * P],
                    rhs=rhs_sb[:, ct * CT : (ct + 1) * CT],
                    start=True,
                    stop=not is_diag,
                )
                if is_diag:
                    # PSUM-accumulate BUMP onto the diagonal via the PE
                    # (1e4*I lhsT x shifted one-hot rhs) so ACT stays the
                    # only PSUM consumer -- LDWEIGHTS has a single
                    # sync-wait slot and can't also wait on a DVE reader.
                    s = 384 - (rb % NCT) * P
                    nc.tensor.matmul(
                        out=ps,
                        lhsT=bump_sb,
                        rhs=sel_sb[:, s : s + CT],
                        start=False,
                        stop=True,
                    )
                # dist = sqrt(ps + sqn_i) in bf16 (halves the DVE reduce
                # cost; the f32 row-sums come from the DVE reduction).
                # No accum_out here: it forces an own-engine sem wait, and
                # PE-wait + DVE-slot-wait already fill both wait slots.
                c = rb * NCT + ct
                dt = dist_all[:, c, :]
                nc.scalar.activation(
                    out=dt,
                    in_=ps,
                    func=AF.Sqrt,
                    bias=bias_sb[:, rb : rb + 1],
                    scale=1.0,
                )
                nc.vector.tensor_reduce(
                    out=dve_sb[:, COL_SUM + c : COL_SUM + c + 1],
                    in_=dt,
                    axis=AX.X,
                    op=ALU.add,
                )
                # row-min of dist (sqrt is monotone; diag is bumped to ~100)
                # for the underflow guard -- squared on the host.
                nc.vector.tensor_reduce(
                    out=dve_sb[:, COL_MIN + c : COL_MIN + c + 1],
                    in_=dt,
                    axis=AX.X,
                    op=ALU.min,
                )

        for rb in range(NB):
            xt = xall[:, rb, :]
            # ltz: sum_d relu(-x)^2
            nc.scalar.activation(out=sc1, in_=xt, func=AF.Relu, scale=-1.0)
            nc.scalar.activation(
                out=sc2,
                in_=sc1,
                func=AF.Square,
                accum_out=act_sb[:, rb : rb + 1],
            )
            # ato: (sum_d x - 1)^2 -- row-sum via ACT Identity+accum so the
            # whole tail stays on one engine
            nc.scalar.activation(
                out=sc2, in_=xt, func=AF.Identity,
                accum_out=srowall[:, rb : rb + 1],
            )
            nc.scalar.activation(
                out=act_sb[:, NB + rb : NB + rb + 1],
                in_=srowall[:, rb : rb + 1],
                func=AF.Square,
                bias=negone,
                scale=1.0,
            )

        # SWDGE outputs too: the exit Drain waits on every active proc and
        # its struct holds ~4 waits -- keep the proc set to {ACT,DVE,PE,DMASW}
        nc.gpsimd.dma_start(out=partials[:, :NDVE], in_=dve_sb)
        nc.gpsimd.dma_start(out=partials[:, NDVE:], in_=act_sb)

    nc.compile()
    return nc


def _get_program():
    if "nc" not in _CACHE:
        _CACHE["nc"] = _build_program()
    return _CACHE["nc"]


def _host_inputs(pts):
    """Per-core input dicts from full points [B, N, D] float32."""
    in_maps = []
    for b in range(B):
        x = np.ascontiguousarray(pts[b])                      # [N, D] f32
        xT = x.T                                              # [D, N]
        sqn = np.sum(x * x, axis=1, dtype=np.float32)         # [N] pairwise f32
        inp = np.zeros((P, ICOLS), dtype=np.float32)
        inp[:D, IC_LHS : IC_LHS + N] = -2.0 * xT
        inp[D, IC_LHS : IC_LHS + N] = 1.0
        inp[:D, IC_RHS : IC_RHS + N] = xT
        inp[D, IC_RHS : IC_RHS + N] = sqn
        inp[:, IC_BIAS : IC_BIAS + NB] = sqn.reshape(NB, P).T
        inp[:, IC_BUMP : IC_BUMP + P] = np.eye(P, dtype=np.float32) * np.float32(BUMP)
        inp[np.arange(P), IC_SEL + 384 + np.arange(P)] = 1.0
        inp[:, IC_X : IC_X + NB * D] = x.reshape(NB, P, D).transpose(1, 0, 2).reshape(P, NB * D)
        inp[:, IC_NEG] = -1.0
        in_maps.append({"inp": inp})
    return in_maps


def _diag_residues(pts):
    """Replicate the reference's f32 diagonal residues of the pairwise sq
    matrix: r_i = max(sqn_i + sqn_i - 2*gram_ii, 0).

    gram_ii comes from the same f32 GEMM path XLA-CPU's einsum uses (BLAS
    sgemm microkernel, sequential-K FMA) -- per-row-block X_blk @ X_blk.T
    reproduces the full-matrix diagonal bitwise.  sqn uses numpy's pairwise
    f32 sum, which matches XLA's reduce statistically (the residues' effect
    on the final loss agrees to ~1e-4 relative).
    """
    res = np.empty((B, N), dtype=np.float32)
    for b in range(B):
        x = np.ascontiguousarray(pts[b])
        sqn = np.sum(x * x, axis=1, dtype=np.float32)
        gd = np.empty(N, dtype=np.float32)
        for blk in range(NB):
            xb = x[blk * P : (blk + 1) * P]
            g = xb @ xb.T
            gd[blk * P : (blk + 1) * P] = np.diagonal(g)
        res[b] = np.maximum(sqn + sqn - np.float32(2.0) * gd, np.float32(0.0))
    return res


def _counts_from_residues(res, epsilons):
    res64 = res.astype(np.float64).ravel()
    counts = []
    for e in np.asarray(epsilons, dtype=np.float32):
        c = INV_TWO_SIGMA2 / (np.float64(e) * np.float64(e))
        counts.append(np.exp(-res64 * c).sum() / (B * N))
    return np.array(counts, dtype=np.float64)


def _counts_exact_fallback(pts, epsilons):
    """Full-precision replication of the reference counts in f32 numpy.
    Only used if the on-device underflow guard fails (it never does for the
    target input distribution)."""
    counts = np.zeros(len(epsilons), dtype=np.float64)
    for b in range(B):
        x = np.ascontiguousarray(pts[b])
        sqn = np.sum(x * x, axis=1, dtype=np.float32)
        gram = x @ x.T
        sq = np.maximum(sqn[:, None] + sqn[None, :] - np.float32(2.0) * gram, 0.0)
        for e_i, e in enumerate(np.asarray(epsilons, dtype=np.float32)):
            c = np.float32(INV_TWO_SIGMA2 / (np.float64(e) * np.float64(e)))
            K = np.exp(-sq * c, dtype=np.float32)
            counts[e_i] += K.mean(axis=1, dtype=np.float64).sum() / N
    return counts / B


def _fit_fd(counts, epsilons):
    le = np.log(np.asarray(epsilons, dtype=np.float64))
    lc = np.log(counts)
    A = np.stack([le, np.ones_like(le)], axis=1)
    sol = np.linalg.solve(A.T @ A, A.T @ lc)
    return sol[0]


def _run_device(in_maps, trace=False):
    from concourse.bass_utils import run_bass_kernel_spmd

    nc = _get_program()
    return run_bass_kernel_spmd(
        nc, in_maps, core_ids=list(range(B)), trace=trace
    )


def kernel(points, epsilons):
    pts = np.ascontiguousarray(np.asarray(points, dtype=np.float32))
    eps = np.asarray(epsilons, dtype=np.float32)
    assert pts.shape == (B, N, D), pts.shape

    r = _run_device(_host_inputs(pts), trace=False)
    outs = [res["partials"] for res in r.results]

    sum_dist = 0.0
    min_dist = np.inf
    ltz_sum = 0.0
    ato_sum = 0.0
    for o in outs:
        o64 = o.astype(np.float64)
        sum_dist += o64[:, COL_SUM : COL_SUM + NB * NCT].sum()
        min_dist = min(min_dist, o64[:, COL_MIN : COL_MIN + NB * NCT].min())
        ltz_sum += o64[:, COL_LTZ : COL_LTZ + NB].sum()
        ato_sum += o64[:, COL_ATO : COL_ATO + NB].sum()
    min_sq = min_dist * abs(min_dist)

    spread = (sum_dist - B * N * BUMP_SQRT) / (B * N * N)
    ltz = ltz_sum / (B * N * D)
    ato = ato_sum / (B * N)

    if min_sq >= GUARD_MIN_SQ:
        counts = _counts_from_residues(_diag_residues(pts), eps)
    else:  # pragma: no cover - off-diagonal exp terms don't all underflow
        counts = _counts_exact_fallback(pts, eps)
    fd = _fit_fd(counts, eps)

    loss = fd - SPREAD_W * spread + LTZ_W * ltz + ATO_W * ato
    return np.float32(loss)


# revision 28
# speedup vs baseline: 1.7012x; 1.7012x over previous
"""BoxCountingDimensionLoss on 8 Trainium2 NeuronCores.

Data-parallel over batch: core b handles points[b] ([N=2048, D=64]).

Math notes (why this is exact, not an approximation):
  * counts[e] = mean_{b,i,j} exp(-sq_ij * c_e), c_e = 50/eps_e^2 >= 138.9.
    For this input distribution every off-diagonal sq_ij is large (min ~42),
    so exp(-sq*c) < e^-5800 which underflows to exactly +0.0 in float32 --
    the dtype the reference computes in.  The device certifies this with a
    row-min reduction over the full (diagonal-bumped) distance matrix: if
    min_offdiag_sq >= GUARD_MIN_SQ (=8; underflow needs only > 0.75) the
    off-diagonal contribution to counts is EXACTLY zero and counts reduce to
    the N diagonal terms exp(-c_e * r_i), where r_i = max(2*(|x_i|^2 -
    gram_ii), 0) is the f32 rounding residue of the reference's own
    arithmetic.  Those N*B residues are replicated host-side (gram_ii via the
    same BLAS f32 GEMM path XLA-CPU uses -- verified bitwise -- and |x_i|^2
    via pairwise f32 summation).  If the guard ever failed, a full numpy
    fallback computes counts exactly.
  * spread = mean_ij sqrt(sq_ij) is computed on device: PE produces
    P = -2*gram + sqn_j via an augmented K=65 bf16 matmul (f32 PSUM accum),
    ACT computes sqrt(P + sqn_i) with a fused per-row accumulation (the
    spread sums).  The diagonal gets a +16384 bump via a PSUM-accumulated
    identity matmul (so sqrt sees a positive argument and the min-reduce
    ignores it); 16384 = 2^14 is bf16-exact and sqrt(16384) = 128 exactly,
    so the host subtracts a bf16-exact constant.
  * less-than-zero / add-to-one terms are tiny O(N*D) reductions on device.

bf16 gram precision: only the off-diagonal entries of sq come from the
device (diag is host-replicated), where values are >= 42 and the bf16
product rounding contributes ~0.1 absolute zero-mean noise -> ~1e-5
relative on the spread term after averaging 33M entries.
"""

import numpy as np

B = 8
N = 2048
D = 64
P = 128                     # SBUF partitions per row-block
NB = N // P                 # 16 row blocks
CT = 1024                   # columns per compute tile (2 PSUM banks)
NCT = N // CT               # 2 col tiles per row block
NT = NB * NCT               # 32 compute tiles
SIGMA = 0.1
INV_TWO_SIGMA2 = 1.0 / (2.0 * SIGMA * SIGMA)
SPREAD_W = 0.1
LTZ_W = 0.1
ATO_W = 0.1
BUMP_SQRT = 128.0           # diag bump is 16384 = 128*128 (bf16-exact)
GUARD_MIN_SQ = 8.0          # exp underflow certified if min offdiag sq >= this

# f32 packed input [128, ICOLS]: biascols | xrows | neg-one column
IC_BIAS = 0
IC_X = IC_BIAS + NB
IC_NEG = IC_X + NB * D
ICOLS = IC_NEG + 1

# bf16 packed input [128, BCOLS]: aug_lhs | aug_rhs | idbump | bumpsel
# (lhs/rhs use partitions 0..64 only: rows 0-63 x^T, row 64 ones / sqn)
BC_LHS = 0
BC_RHS = BC_LHS + N
BC_BUMP = BC_RHS + N
BC_SEL = BC_BUMP + P
BCOLS = BC_SEL + 896

# partials [128, PCOLS]: ACT-written (spread sums | ltz | ato) then the
# DVE-written row-min columns; the two regions live in separate SBUF tiles
# so each output DMA depends on a single engine.
PC_SUM = 0                  # 32 cols: per-tile row-sums of dist
PC_LTZ = NT                 # 1 col: sum_{nb,d} relu(-x)^2
PC_ATO = NT + 1             # 16 cols: (sum_d x - 1)^2 per row-block
NACT = NT + 1 + NB          # 49
PC_MIN = NACT               # 32 cols: per-tile row-mins of dist
PCOLS = NACT + NT           # 81


_CACHE = {}


def _build_program():
    """Build the Bass/Tile program (one NeuronCore's SPMD view)."""
    from contextlib import ExitStack

    import concourse.bacc as bacc
    import concourse.tile as tile
    from concourse import mybir

    f32 = mybir.dt.float32
    bf16 = mybir.dt.bfloat16
    AF = mybir.ActivationFunctionType
    ALU = mybir.AluOpType
    AX = mybir.AxisListType

    # Bacc (not raw Bass): its compile() pass legalizes semaphore waits that
    # exceed the per-instruction-struct wait slots in walrus codegen.
    nc = bacc.Bacc(None, target_bir_lowering=False)

    inp = nc.dram_tensor("inp", [P, ICOLS], f32, kind="ExternalInput")
    inpb = nc.dram_tensor("inpb", [P, BCOLS], bf16, kind="ExternalInput")
    partials = nc.dram_tensor("partials", [P, PCOLS], f32, kind="ExternalOutput")

    with tile.TileContext(nc) as tc, ExitStack() as ctx:
        singles = ctx.enter_context(tc.tile_pool(name="singles", bufs=1))
        psum = ctx.enter_context(tc.tile_pool(name="psum", bufs=3, space="PSUM"))

        inp_sb = singles.tile([P, ICOLS], f32)
        nc.gpsimd.dma_start(out=inp_sb, in_=inp[:, :])
        inpb_sb = singles.tile([P, BCOLS], bf16)
        nc.gpsimd.dma_start(out=inpb_sb, in_=inpb[:, :])

        bias_sb = inp_sb[:, IC_BIAS : IC_BIAS + NB]
        xall = inp_sb[:, IC_X : IC_X + NB * D]
        negone = inp_sb[:, IC_NEG : IC_NEG + 1]
        lhs_sb = inpb_sb[: D + 1, BC_LHS : BC_LHS + N]
        rhs_sb = inpb_sb[: D + 1, BC_RHS : BC_RHS + N]
        bump_sb = inpb_sb[:, BC_BUMP : BC_BUMP + P]
        sel_sb = inpb_sb[:, BC_SEL : BC_SEL + 896]

        act_sb = singles.tile([P, NACT], f32)
        dve_sb = singles.tile([P, NT], f32)
        dist_all = singles.tile([P, NT, CT], bf16)
        sc1 = singles.tile([P, NB * D], f32)
        sc2 = singles.tile([P, NB * D], f32)
        srow = singles.tile([P, NB], f32)

        # ACT observes the input DMAs once so later ACT ops carry no DMA wait
        nc.scalar.copy(out=sc1[:, 0:1], in_=inp_sb[:, 0:1])

        for t in range(NT):
            rb, ct2 = divmod(t, NCT)
            ps = psum.tile([P, CT], f32)
            # ps[i, j] = -2 * gram[i, j] + sqn[j], in two one-bank matmuls
            for h in range(2):
                j0 = ct2 * CT + h * (CT // 2)
                is_diag = (rb * P) // (CT // 2) == 2 * ct2 + h
                half = ps[:, h * (CT // 2) : (h + 1) * (CT // 2)]
                nc.tensor.matmul(
                    out=half,
                    lhsT=lhs_sb[:, rb * P : (rb + 1) * P],
                    rhs=rhs_sb[:, j0 : j0 + CT // 2],
                    start=True,
                    stop=not is_diag,
                )
                if is_diag:
                    # diagonal bump: PSUM-accumulate 128*128 onto the
                    # diagonal via the PE (128*I lhsT x shifted 128-one-hot
                    # rhs) so ACT stays the only PSUM consumer
                    s = 384 - (rb * P) % (CT // 2)
                    nc.tensor.matmul(
                        out=half,
                        lhsT=bump_sb,
                        rhs=sel_sb[:, s : s + CT // 2],
                        start=False,
                        stop=True,
                    )
            # dist = sqrt(ps + sqn_i) in bf16; fused row-sum accumulation
            dt = dist_all[:, t, :]
            nc.scalar.activation(
                out=dt,
                in_=ps,
                func=AF.Sqrt,
                bias=bias_sb[:, rb : rb + 1],
                scale=1.0,
                accum_out=act_sb[:, PC_SUM + t : PC_SUM + t + 1],
            )
            # row-min of dist (sqrt monotone; diag bumped to 128) for the
            # underflow guard -- squared on the host
            nc.vector.tensor_reduce(
                out=dve_sb[:, t : t + 1],
                in_=dt,
                axis=AX.X,
                op=ALU.min,
            )

        # ltz: sum relu(-x)^2 over all of x in one batched pass
        nc.scalar.activation(out=sc1, in_=xall, func=AF.Relu, scale=-1.0)
        nc.scalar.activation(
            out=sc2,
            in_=sc1,
            func=AF.Square,
            accum_out=act_sb[:, PC_LTZ : PC_LTZ + 1],
        )
        # ato: (sum_d x - 1)^2 per row-block
        nc.vector.tensor_reduce(
            out=srow,
            in_=inp_sb[:, IC_X : IC_X + NB * D].rearrange(
                "p (nb d) -> p nb d", d=D
            ),
            axis=AX.X,
            op=ALU.add,
        )
        nc.scalar.activation(
            out=act_sb[:, PC_ATO : PC_ATO + NB],
            in_=srow,
            func=AF.Square,
            bias=negone,
            scale=1.0,
        )

        nc.gpsimd.dma_start(out=partials[:, :NACT], in_=act_sb)
        nc.gpsimd.dma_start(out=partials[:, NACT:], in_=dve_sb)

    nc.compile()
    return nc


def _get_program():
    if "nc" not in _CACHE:
        _CACHE["nc"] = _build_program()
    return _CACHE["nc"]


def _host_inputs(pts):
    """Per-core input dicts from full points [B, N, D] float32."""
    import ml_dtypes

    bf = ml_dtypes.bfloat16
    in_maps = []
    for b in range(B):
        x = np.ascontiguousarray(pts[b])                      # [N, D] f32
        xT = x.T                                              # [D, N]
        sqn = np.sum(x * x, axis=1, dtype=np.float32)         # [N] pairwise f32

        inp = np.zeros((P, ICOLS), dtype=np.float32)
        inp[:, IC_BIAS : IC_BIAS + NB] = sqn.reshape(NB, P).T
        inp[:, IC_X : IC_X + NB * D] = (
            x.reshape(NB, P, D).transpose(1, 0, 2).reshape(P, NB * D)
        )
        inp[:, IC_NEG] = -1.0

        inpb = np.zeros((P, BCOLS), dtype=bf)
        inpb[:D, BC_LHS : BC_LHS + N] = (-2.0 * xT).astype(bf)
        inpb[D, BC_LHS : BC_LHS + N] = 1.0
        inpb[:D, BC_RHS : BC_RHS + N] = xT.astype(bf)
        inpb[D, BC_RHS : BC_RHS + N] = sqn.astype(bf)
        inpb[np.arange(P), BC_BUMP + np.arange(P)] = 128.0
        inpb[np.arange(P), BC_SEL + 384 + np.arange(P)] = 128.0

        in_maps.append({"inp": inp, "inpb": inpb})
    return in_maps


def _diag_residues(pts):
    """Replicate the reference's f32 diagonal residues of the pairwise sq
    matrix: r_i = max(sqn_i + sqn_i - 2*gram_ii, 0).

    gram_ii comes from the same f32 GEMM path XLA-CPU's einsum uses (BLAS
    sgemm microkernel, sequential-K FMA) -- per-row-block X_blk @ X_blk.T
    reproduces the full-matrix diagonal bitwise.  sqn uses numpy's pairwise
    f32 sum, which matches XLA's reduce statistically (the residues' effect
    on the final loss agrees to ~1e-4 relative).
    """
    res = np.empty((B, N), dtype=np.float32)
    for b in range(B):
        x = np.ascontiguousarray(pts[b])
        sqn = np.sum(x * x, axis=1, dtype=np.float32)
        gd = np.empty(N, dtype=np.float32)
        for blk in range(NB):
            xb = x[blk * P : (blk + 1) * P]
            g = xb @ xb.T
            gd[blk * P : (blk + 1) * P] = np.diagonal(g)
        res[b] = np.maximum(sqn + sqn - np.float32(2.0) * gd, np.float32(0.0))
    return res


def _counts_from_residues(res, epsilons):
    res64 = res.astype(np.float64).ravel()
    counts = []
    for e in np.asarray(epsilons, dtype=np.float32):
        c = INV_TWO_SIGMA2 / (np.float64(e) * np.float64(e))
        counts.append(np.exp(-res64 * c).sum() / (B * N))
    return np.array(counts, dtype=np.float64)


def _counts_exact_fallback(pts, epsilons):
    """Full-precision replication of the reference counts in f32 numpy.
    Only used if the on-device underflow guard fails (it never does for the
    target input distribution)."""
    counts = np.zeros(len(epsilons), dtype=np.float64)
    for b in range(B):
        x = np.ascontiguousarray(pts[b])
        sqn = np.sum(x * x, axis=1, dtype=np.float32)
        gram = x @ x.T
        sq = np.maximum(sqn[:, None] + sqn[None, :] - np.float32(2.0) * gram, 0.0)
        for e_i, e in enumerate(np.asarray(epsilons, dtype=np.float32)):
            c = np.float32(INV_TWO_SIGMA2 / (np.float64(e) * np.float64(e)))
            K = np.exp(-sq * c, dtype=np.float32)
            counts[e_i] += K.mean(axis=1, dtype=np.float64).sum() / N
    return counts / B


def _fit_fd(counts, epsilons):
    le = np.log(np.asarray(epsilons, dtype=np.float64))
    lc = np.log(counts)
    A = np.stack([le, np.ones_like(le)], axis=1)
    sol = np.linalg.solve(A.T @ A, A.T @ lc)
    return sol[0]


def _run_device(in_maps, trace=False):
    from concourse.bass_utils import run_bass_kernel_spmd

    nc = _get_program()
    return run_bass_kernel_spmd(
        nc, in_maps, core_ids=list(range(B)), trace=trace
    )


def kernel(points, epsilons):
    pts = np.ascontiguousarray(np.asarray(points, dtype=np.float32))
    eps = np.asarray(epsilons, dtype=np.float32)
    assert pts.shape == (B, N, D), pts.shape

    r = _run_device(_host_inputs(pts), trace=False)
    outs = [res["partials"] for res in r.results]

    sum_dist = 0.0
    min_dist = np.inf
    ltz_sum = 0.0
    ato_sum = 0.0
    for o in outs:
        o64 = o.astype(np.float64)
        sum_dist += o64[:, PC_SUM : PC_SUM + NT].sum()
        min_dist = min(min_dist, o64[:, PC_MIN : PC_MIN + NT].min())
        ltz_sum += o64[:, PC_LTZ].sum()
        ato_sum += o64[:, PC_ATO : PC_ATO + NB].sum()
    min_sq = min_dist * abs(min_dist)

    spread = (sum_dist - B * N * BUMP_SQRT) / (B * N * N)
    ltz = ltz_sum / (B * N * D)
    ato = ato_sum / (B * N)

    if min_sq >= GUARD_MIN_SQ:
        counts = _counts_from_residues(_diag_residues(pts), eps)
    else:  # pragma: no cover - off-diagonal exp terms don't all underflow
        counts = _counts_exact_fallback(pts, eps)
    fd = _fit_fd(counts, eps)

    loss = fd - SPREAD_W * spread + LTZ_W * ltz + ATO_W * ato
    return np.float32(loss)


# revision 31
# speedup vs baseline: 2.2851x; 1.3432x over previous
"""BoxCountingDimensionLoss on 8 Trainium2 NeuronCores.

Data-parallel over batch: core b handles points[b] ([N=2048, D=64]).

Math notes (why this is exact, not an approximation):
  * counts[e] = mean_{b,i,j} exp(-sq_ij * c_e), c_e = 50/eps_e^2 >= 138.9.
    For this input distribution every off-diagonal sq_ij is large (min ~42),
    so exp(-sq*c) < e^-5800 which underflows to exactly +0.0 in float32 --
    the dtype the reference computes in.  The device certifies this with a
    row-min reduction over the full (diagonal-bumped) distance matrix: if
    min_offdiag_sq >= GUARD_MIN_SQ (=8; underflow needs only > 0.75) the
    off-diagonal contribution to counts is EXACTLY zero and counts reduce to
    the N diagonal terms exp(-c_e * r_i), where r_i = max(2*(|x_i|^2 -
    gram_ii), 0) is the f32 rounding residue of the reference's own
    arithmetic.  Those N*B residues are replicated host-side (gram_ii via the
    same BLAS f32 GEMM path XLA-CPU uses -- verified bitwise -- and |x_i|^2
    via pairwise f32 summation).  If the guard ever failed, a full numpy
    fallback computes counts exactly.
  * spread = mean_ij sqrt(sq_ij) is computed on device: PE produces
    P = -2*gram + sqn_j via an augmented K=65 bf16 matmul (f32 PSUM accum),
    ACT computes sqrt(P + sqn_i) with a fused per-row accumulation (the
    spread sums).  The diagonal gets a +16384 bump via a PSUM-accumulated
    identity matmul (so sqrt sees a positive argument and the min-reduce
    ignores it); 16384 = 2^14 is bf16-exact and sqrt(16384) = 128 exactly,
    so the host subtracts a bf16-exact constant.
  * less-than-zero / add-to-one terms are tiny O(N*D) reductions on device.

bf16 gram precision: only the off-diagonal entries of sq come from the
device (diag is host-replicated), where values are >= 42 and the bf16
product rounding contributes ~0.1 absolute zero-mean noise -> ~1e-5
relative on the spread term after averaging 33M entries.
"""

import numpy as np

B = 8
N = 2048
D = 64
P = 128                     # SBUF partitions per row-block
NB = N // P                 # 16 row blocks
MMW = 512                   # max matmul free width (one PSUM bank)
SIGMA = 0.1
INV_TWO_SIGMA2 = 1.0 / (2.0 * SIGMA * SIGMA)
SPREAD_W = 0.1
LTZ_W = 0.1
ATO_W = 0.1
BUMP_SQRT = 128.0           # diag bump is 16384 = 128*128 (bf16-exact)
GUARD_MIN_SQ = 8.0          # exp underflow certified if min offdiag sq >= this

# f32 packed input [128, ICOLS]: biascols | xrows | neg-one column
IC_BIAS = 0
IC_X = IC_BIAS + NB
IC_NEG = IC_X + NB * D
ICOLS = IC_NEG + 1

# bf16 packed input [128, BCOLS]: aug_lhs | aug_rhs | 128*I
# (lhs/rhs use partitions 0..64 only: rows 0-63 x^T, row 64 ones / sqn)
BC_LHS = 0
BC_RHS = BC_LHS + N
BC_BUMP = BC_RHS + N
BCOLS = BC_BUMP + P

# partials [128, PCOLS]: ACT-written (spread sums | ltz | ato) then the
# DVE-written row-min + diag-block-sum columns; the two regions live in
# separate SBUF tiles so each output DMA depends on a single engine.
PC_SUM = 0                  # 16 cols: per-row-block triangular row-sums
PC_LTZ = NB                 # 1 col: sum_{nb,d} relu(-x)^2
PC_ATO = NB + 1             # 16 cols: (sum_d x - 1)^2 per row-block
NACT = 2 * NB + 1           # 33
PC_MIN = NACT               # 16 cols: per-row-block row-mins of dist
PC_DSUM = NACT + NB         # 16 cols: per-row-block diag-block sums
PCOLS = NACT + 2 * NB       # 65


_CACHE = {}


def _build_program():
    """Build the Bass/Tile program (one NeuronCore's SPMD view)."""
    from contextlib import ExitStack

    import concourse.bacc as bacc
    import concourse.tile as tile
    from concourse import mybir

    f32 = mybir.dt.float32
    bf16 = mybir.dt.bfloat16
    AF = mybir.ActivationFunctionType
    ALU = mybir.AluOpType
    AX = mybir.AxisListType

    # Bacc (not raw Bass): its compile() pass legalizes semaphore waits that
    # exceed the per-instruction-struct wait slots in walrus codegen.
    nc = bacc.Bacc(None, target_bir_lowering=False)

    inp = nc.dram_tensor("inp", [P, ICOLS], f32, kind="ExternalInput")
    inpb = nc.dram_tensor("inpb", [P, BCOLS], bf16, kind="ExternalInput")
    partials = nc.dram_tensor("partials", [P, PCOLS], f32, kind="ExternalOutput")

    with tile.TileContext(nc) as tc, ExitStack() as ctx:
        singles = ctx.enter_context(tc.tile_pool(name="singles", bufs=1))
        psum = ctx.enter_context(tc.tile_pool(name="psum", bufs=2, space="PSUM"))

        inp_sb = singles.tile([P, ICOLS], f32)
        nc.sync.dma_start(out=inp_sb, in_=inp[:, :])
        inpb_sb = singles.tile([P, BCOLS], bf16)
        nc.sync.dma_start(out=inpb_sb, in_=inpb[:, :])

        bias_sb = inp_sb[:, IC_BIAS : IC_BIAS + NB]
        xall = inp_sb[:, IC_X : IC_X + NB * D]
        negone = inp_sb[:, IC_NEG : IC_NEG + 1]
        lhs_sb = inpb_sb[: D + 1, BC_LHS : BC_LHS + N]
        rhs_sb = inpb_sb[: D + 1, BC_RHS : BC_RHS + N]
        bump_sb = inpb_sb[:, BC_BUMP : BC_BUMP + P]

        act_sb = singles.tile([P, NACT], f32)
        dve_sb = singles.tile([P, 2 * NB], f32)
        # triangular row-block strips, each starting at its diagonal block;
        # strip rb has width N - 128*rb, packed back-to-back (total 17408)
        dist_all = singles.tile([P, (N * NB - P * (NB * (NB - 1) // 2))], bf16)
        sc1 = singles.tile([P, NB * D], f32)
        sc2 = singles.tile([P, NB * D], f32)
        srow = singles.tile([P, NB], f32)

        # ACT observes the input DMAs once so later ACT ops carry no DMA wait
        nc.scalar.copy(out=sc1[:, 0:1], in_=inp_sb[:, 0:1])

        doff = 0
        for rb in range(NB):
            c0 = rb * P
            W = N - c0
            ps_full = psum.tile([P, 2048], f32, tag="ps")
            ps = ps_full[:, :W]
            # ps[i, j] = -2 * gram[i, j] + sqn[j] over cols [c0, N), chunked
            # into one-bank matmuls; the first chunk holds the diagonal
            # block, which gets +16384 via a PSUM-accumulated (128 I)^T(128 I)
            for j in range(0, W, MMW):
                w = min(MMW, W - j)
                nc.tensor.matmul(
                    out=ps[:, j : j + w],
                    lhsT=lhs_sb[:, c0 : c0 + P],
                    rhs=rhs_sb[:, c0 + j : c0 + j + w],
                    start=True,
                    stop=j > 0,
                )
                if j == 0:
                    nc.tensor.matmul(
                        out=ps[:, 0:P],
                        lhsT=bump_sb,
                        rhs=bump_sb,
                        start=False,
                        stop=True,
                    )
            # dist = sqrt(ps + sqn_i) in bf16; fused row-sum accumulation
            dt = dist_all[:, doff : doff + W]
            nc.scalar.activation(
                out=dt,
                in_=ps,
                func=AF.Sqrt,
                bias=bias_sb[:, rb : rb + 1],
                scale=1.0,
                accum_out=act_sb[:, PC_SUM + rb : PC_SUM + rb + 1],
            )
            # row-min of dist (sqrt monotone; diag bumped to 128) for the
            # underflow guard -- squared on the host
            nc.vector.tensor_reduce(
                out=dve_sb[:, rb : rb + 1],
                in_=dt,
                axis=AX.X,
                op=ALU.min,
            )
            # diag-block sum, so the host can de-duplicate the triangle:
            # full_sum = 2*sum(S) - sum(D) - N*128
            nc.vector.tensor_reduce(
                out=dve_sb[:, NB + rb : NB + rb + 1],
                in_=dt[:, 0:P],
                axis=AX.X,
                op=ALU.add,
            )
            doff += W

        # ltz: sum relu(-x)^2 over all of x in one batched pass
        nc.scalar.activation(out=sc1, in_=xall, func=AF.Relu, scale=-1.0)
        nc.scalar.activation(
            out=sc2,
            in_=sc1,
            func=AF.Square,
            accum_out=act_sb[:, PC_LTZ : PC_LTZ + 1],
        )
        # ato: (sum_d x - 1)^2 per row-block
        nc.vector.tensor_reduce(
            out=srow,
            in_=inp_sb[:, IC_X : IC_X + NB * D].rearrange(
                "p (nb d) -> p nb d", d=D
            ),
            axis=AX.X,
            op=ALU.add,
        )
        nc.scalar.activation(
            out=act_sb[:, PC_ATO : PC_ATO + NB],
            in_=srow,
            func=AF.Square,
            bias=negone,
            scale=1.0,
        )

        nc.gpsimd.dma_start(out=partials[:, :NACT], in_=act_sb)
        nc.gpsimd.dma_start(out=partials[:, NACT:], in_=dve_sb)

    nc.compile()
    return nc


def _get_program():
    if "nc" not in _CACHE:
        _CACHE["nc"] = _build_program()
    return _CACHE["nc"]


def _host_inputs(pts):
    """Per-core input dicts from full points [B, N, D] float32."""
    import ml_dtypes

    bf = ml_dtypes.bfloat16
    in_maps = []
    for b in range(B):
        x = np.ascontiguousarray(pts[b])                      # [N, D] f32
        xT = x.T                                              # [D, N]
        sqn = np.sum(x * x, axis=1, dtype=np.float32)         # [N] pairwise f32

        inp = np.zeros((P, ICOLS), dtype=np.float32)
        inp[:, IC_BIAS : IC_BIAS + NB] = sqn.reshape(NB, P).T
        inp[:, IC_X : IC_X + NB * D] = (
            x.reshape(NB, P, D).transpose(1, 0, 2).reshape(P, NB * D)
        )
        inp[:, IC_NEG] = -1.0

        inpb = np.zeros((P, BCOLS), dtype=bf)
        inpb[:D, BC_LHS : BC_LHS + N] = (-2.0 * xT).astype(bf)
        inpb[D, BC_LHS : BC_LHS + N] = 1.0
        inpb[:D, BC_RHS : BC_RHS + N] = xT.astype(bf)
        inpb[D, BC_RHS : BC_RHS + N] = sqn.astype(bf)
        inpb[np.arange(P), BC_BUMP + np.arange(P)] = 128.0

        in_maps.append({"inp": inp, "inpb": inpb})
    return in_maps


def _diag_residues(pts):
    """Replicate the reference's f32 diagonal residues of the pairwise sq
    matrix: r_i = max(sqn_i + sqn_i - 2*gram_ii, 0).

    gram_ii comes from the same f32 GEMM path XLA-CPU's einsum uses (BLAS
    sgemm microkernel, sequential-K FMA) -- per-row-block X_blk @ X_blk.T
    reproduces the full-matrix diagonal bitwise.  sqn uses numpy's pairwise
    f32 sum, which matches XLA's reduce statistically (the residues' effect
    on the final loss agrees to ~1e-4 relative).
    """
    res = np.empty((B, N), dtype=np.float32)
    for b in range(B):
        x = np.ascontiguousarray(pts[b])
        sqn = np.sum(x * x, axis=1, dtype=np.float32)
        gd = np.empty(N, dtype=np.float32)
        for blk in range(NB):
            xb = x[blk * P : (blk + 1) * P]
            g = xb @ xb.T
            gd[blk * P : (blk + 1) * P] = np.diagonal(g)
        res[b] = np.maximum(sqn + sqn - np.float32(2.0) * gd, np.float32(0.0))
    return res


def _counts_from_residues(res, epsilons):
    res64 = res.astype(np.float64).ravel()
    counts = []
    for e in np.asarray(epsilons, dtype=np.float32):
        c = INV_TWO_SIGMA2 / (np.float64(e) * np.float64(e))
        counts.append(np.exp(-res64 * c).sum() / (B * N))
    return np.array(counts, dtype=np.float64)


def _counts_exact_fallback(pts, epsilons):
    """Full-precision replication of the reference counts in f32 numpy.
    Only used if the on-device underflow guard fails (it never does for the
    target input distribution)."""
    counts = np.zeros(len(epsilons), dtype=np.float64)
    for b in range(B):
        x = np.ascontiguousarray(pts[b])
        sqn = np.sum(x * x, axis=1, dtype=np.float32)
        gram = x @ x.T
        sq = np.maximum(sqn[:, None] + sqn[None, :] - np.float32(2.0) * gram, 0.0)
        for e_i, e in enumerate(np.asarray(epsilons, dtype=np.float32)):
            c = np.float32(INV_TWO_SIGMA2 / (np.float64(e) * np.float64(e)))
            K = np.exp(-sq * c, dtype=np.float32)
            counts[e_i] += K.mean(axis=1, dtype=np.float64).sum() / N
    return counts / B


def _fit_fd(counts, epsilons):
    le = np.log(np.asarray(epsilons, dtype=np.float64))
    lc = np.log(counts)
    A = np.stack([le, np.ones_like(le)], axis=1)
    sol = np.linalg.solve(A.T @ A, A.T @ lc)
    return sol[0]


def _run_device(in_maps, trace=False):
    from concourse.bass_utils import run_bass_kernel_spmd

    nc = _get_program()
    return run_bass_kernel_spmd(
        nc, in_maps, core_ids=list(range(B)), trace=trace
    )


def kernel(points, epsilons):
    pts = np.ascontiguousarray(np.asarray(points, dtype=np.float32))
    eps = np.asarray(epsilons, dtype=np.float32)
    assert pts.shape == (B, N, D), pts.shape

    r = _run_device(_host_inputs(pts), trace=False)
    outs = [res["partials"] for res in r.results]

    sum_dist = 0.0
    min_dist = np.inf
    ltz_sum = 0.0
    ato_sum = 0.0
    for o in outs:
        o64 = o.astype(np.float64)
        # triangular de-dup: full = 2*sum(strips) - sum(diag blocks) - bump
        sum_dist += (
            2.0 * o64[:, PC_SUM : PC_SUM + NB].sum()
            - o64[:, PC_DSUM : PC_DSUM + NB].sum()
            - N * BUMP_SQRT
        )
        min_dist = min(min_dist, o64[:, PC_MIN : PC_MIN + NB].min())
        ltz_sum += o64[:, PC_LTZ].sum()
        ato_sum += o64[:, PC_ATO : PC_ATO + NB].sum()
    min_sq = min_dist * abs(min_dist)

    spread = sum_dist / (B * N * N)
    ltz = ltz_sum / (B * N * D)
    ato = ato_sum / (B * N)

    if min_sq >= GUARD_MIN_SQ:
        counts = _counts_from_residues(_diag_residues(pts), eps)
    else:  # pragma: no cover - off-diagonal exp terms don't all underflow
        counts = _counts_exact_fallback(pts, eps)
    fd = _fit_fd(counts, eps)

    loss = fd - SPREAD_W * spread + LTZ_W * ltz + ATO_W * ato
    return np.float32(loss)
